# revision 3
# baseline (speedup 1.0000x reference)
"""DecomposedEmbedding lookup on 8 trn2 NeuronCores.

weight = sw * sigmoid(mask)[:,None] + aw + sum_k(atten[k] * from_kb[...,k]);
out = weight[input_ids].

Strategy (tensor parallel on the embedding table, per the vocab-sharding
hint): the host packs the four vocab tables into one combined bf16 table
comb[v] = [sw | aw | from_kb[...,0] | from_kb[...,1] | mask | pad]
(row stride 1280B, 1028B of payload) and shards it row-wise across the 8
cores (62500 rows each).  Tokens are bucketed by owning core and by
31250-row half-shard (so shard-local indices fit the gather engine's int16
index type).  Each core runs batched DMA-gathers (<=1024 indices each, a
hardware limit) pulling only the rows it needs; the gathers are spread
round-robin over 4 SWDGE queues because the gather engine is
descriptor-rate-bound (~12ns/row/queue), not bandwidth-bound.  Gathered
rows are combined on-chip with fused scalar_tensor_tensor ops on DVE
(sigmoid + per-row scale + three additions), accumulating to f32 for the
output DMA.  The host scatters each core's result rows back into token
order (the inverse bucketing permutation), which replaces the all-to-all
since the full output is assembled on host anyway.
"""

import os

import numpy as np

V = 500000
D = 128
K = 2
NCORES = 8
VS = V // NCORES  # rows per core
HALF = VS // 2  # rows per half-shard (int16-indexable)
P = 128
QB = 1024  # max indices per dma_gather instruction

LAST_EXEC_TIME_NS = None
LAST_RESULTS = None

_PROG_CACHE = {}


def _dt_conf():
    """(mybir dtype name, row stride in elems, gathered elems, mask offset)"""
    if os.environ.get("KDT", "bf16") == "f32":
        return "float32", 576, 514, 512
    return "bfloat16", 640, 514, 512


def _block_sizes(caph):
    """Split caph tokens into near-equal blocks of <=QB, multiples of 128."""
    qb = int(os.environ.get("KBLK", str(QB)))
    nblk = -(-caph // qb)
    g = caph // P
    gper, rem = divmod(g, nblk)
    return [(gper + (1 if i < rem else 0)) * P for i in range(nblk)]


def _build_program(caph):
    from concourse import bacc, tile
    import concourse.mybir as mybir

    kloop = int(os.environ.get("KLOOP", "0"))
    no_gather = os.environ.get("KNO_GATHER", "") == "1"
    no_compute = os.environ.get("KNO_COMPUTE", "") == "1"
    no_out = os.environ.get("KNO_OUT", "") == "1"
    bufs = int(os.environ.get("KBUFS", "3"))
    nq = int(os.environ.get("KQ", "4"))
    dtname, rowe, egath, mask_off = _dt_conf()
    eg = int(os.environ.get("KELEM", str(egath)))

    f32, i16 = mybir.dt.float32, mybir.dt.int16
    cdt = getattr(mybir.dt, dtname)
    cdt_size = mybir.dt.size(cdt)
    nc = bacc.Bacc(
        "TRN2",
        target_bir_lowering=False,
        debug=False,
        enable_asserts=False,
        num_devices=NCORES,
        num_swdge_queues=nq,
    )
    comb = nc.dram_tensor("comb", [VS, rowe], cdt, kind="ExternalInput")
    attn = nc.dram_tensor("attn", [P, K], f32, kind="ExternalInput")
    S = caph // 16
    idx = nc.dram_tensor("idx", [2, P, S], i16, kind="ExternalInput")
    out = nc.dram_tensor("out", [2 * caph, D], f32, kind="ExternalOutput")

    mult = mybir.AluOpType.mult
    add = mybir.AluOpType.add

    def compute(gt, rtf, Gb):
        sig = wpool.tile([P, Gb], f32, tag="sig")
        nc.scalar.activation(
            out=sig[:],
            in_=gt[:, :, mask_off],
            func=mybir.ActivationFunctionType.Sigmoid,
        )
        rt = wpool.tile([P, Gb, D], cdt, tag="rt")
        # rt[:, g, :] = sw_row * sigmoid(mask_row) + aw_row: per-partition
        # scalar multiply fused with the aw addition (scalar_tensor_tensor)
        for g in range(Gb):
            nc.vector.scalar_tensor_tensor(
                out=rt[:, g, :],
                in0=gt[:, g, 0:D],
                scalar=sig[:, g : g + 1],
                in1=gt[:, g, D : 2 * D],
                op0=mult,
                op1=add,
            )
        # += atten[k] * from_kb[..., k], fused multiply-add per term;
        # the last one accumulates to f32 for the output DMA
        rt2 = wpool.tile([P, Gb, D], cdt, tag="rt2")
        nc.vector.scalar_tensor_tensor(
            out=rt2[:],
            in0=gt[:, :, 2 * D : 3 * D],
            scalar=attn_t[:, 0:1],
            in1=rt[:],
            op0=mult,
            op1=add,
        )
        nc.vector.scalar_tensor_tensor(
            out=rtf[:],
            in0=gt[:, :, 3 * D : 4 * D],
            scalar=attn_t[:, 1:2],
            in1=rt2[:],
            op0=mult,
            op1=add,
        )

    def raw_gather(out_ap, in_ap, idxs_ap, num_idxs, elem_size, elem_step, queue_num):
        # bass.dma_gather minus the elem_size%256 assert (a transpose-path
        # restriction); row stride must still be a multiple of 256B.
        g = nc.gpsimd
        stride_bytes = elem_step * cdt_size
        assert stride_bytes % 256 == 0
        _in_ap = g.lower_ap_dma(in_ap, for_custom_bir_dma=True)
        _idxs_ap = g.lower_ap(idxs_ap)
        _out_ap = g.lower_ap(out_ap)
        return g.add_instruction(
            mybir.InstDMAGatherAnt(
                name=g.bass.get_next_instruction_name(),
                ins=[*_in_ap, _idxs_ap, g.lower_val_access(g.to_reg(num_idxs))],
                outs=[_out_ap],
                transpose=False,
                num_idxs=num_idxs,
                elem_size=elem_size,
                stride_bytes_256=stride_bytes // 256,
                gen_mode=0,
                single_packet=True,
                queue_num=queue_num,
                sbuf_tokens_per_rank=0,
                sbuf_free_dim_per_rank=0,
                sbuf_free_dim_pad_per_rank=0,
                sbuf_byte_offset=0,
            )
        )

    def body():
        # pipeline unit: one <=QB-token gather block (HW limit per dma_gather)
        qn = 0
        for h in range(2):
            idx_t = wpool.tile([P, S], i16, tag="idx")
            nc.sync.dma_start(out=idx_t[:], in_=idx[h])
            b0 = 0
            for nb in _block_sizes(caph):
                Gb = nb // P
                gt = wpool.tile([P, Gb, eg], cdt, tag="gt")
                if not no_gather:
                    raw_gather(
                        out_ap=gt[:],
                        in_ap=comb[h * HALF : (h + 1) * HALF, 0:eg],
                        idxs_ap=idx_t[:, b0 // 16 : (b0 + nb) // 16],
                        num_idxs=nb,
                        elem_size=eg,
                        elem_step=rowe,
                        queue_num=qn % nq,
                    )
                    qn += 1
                rtf = wpool.tile([P, Gb, D], f32, tag="rtf")
                if no_compute:
                    nc.vector.tensor_copy(out=rtf[:], in_=gt[:, :, 0:D])
                else:
                    compute(gt, rtf, Gb)
                if not no_out:
                    nc.sync.dma_start(
                        out=out[h * caph + b0 : h * caph + b0 + nb, :].rearrange(
                            "(g p) d -> p g d", p=P
                        ),
                        in_=rtf[:],
                    )
                b0 += nb

    with tile.TileContext(nc) as tc:
        with (
            tc.tile_pool(name="const", bufs=1) as cpool,
            tc.tile_pool(name="work", bufs=bufs) as wpool,
        ):
            attn_t = cpool.tile([P, K], f32)
            nc.sync.dma_start(out=attn_t[:], in_=attn[:])

            if kloop:
                with tc.For_i(0, kloop, 1):
                    body()
            else:
                body()

    nc.compile()
    return nc


def _pack_idx(loc, caph):
    """int16 local row ids -> [P, caph//16], wrapped-by-16 per QB-token
    gather block, replicated across the 8 groups of 16 partitions."""
    arr = np.zeros(caph, dtype=np.int16)
    arr[: len(loc)] = loc
    cols = []
    b0 = 0
    for nb in _block_sizes(caph):
        blk = arr[b0 : b0 + nb]
        cols.append(blk.reshape(nb // 16, 16).T)
        b0 += nb
    return np.tile(np.concatenate(cols, axis=1), (8, 1))


def _prepare(input_ids, sw, mask, aw, atten, from_kb):
    """Host-side packing: combined table, bucketing, per-core in_maps."""
    import ml_dtypes

    dtname, rowe, egath, mask_off = _dt_conf()
    np_cdt = np.float32 if dtname == "float32" else ml_dtypes.bfloat16

    ids_in = np.asarray(input_ids)
    ids = ids_in.reshape(-1)
    sw = np.asarray(sw, dtype=np.float32)
    aw = np.asarray(aw, dtype=np.float32)
    mask = np.asarray(mask, dtype=np.float32)
    atten = np.asarray(atten, dtype=np.float32)
    from_kb = np.asarray(from_kb, dtype=np.float32)

    comb = np.zeros((V, rowe), dtype=np_cdt)
    comb[:, 0:D] = sw
    comb[:, D : 2 * D] = aw
    comb[:, 2 * D : 3 * D] = from_kb[:, :, 0]
    comb[:, 3 * D : 4 * D] = from_kb[:, :, 1]
    comb[:, mask_off] = mask
    attn_r = np.ascontiguousarray(np.broadcast_to(atten[None, :], (P, K)))

    core_of = ids // VS
    rem = ids - core_of * VS
    half_of = rem // HALF
    loc = (rem - half_of * HALF).astype(np.int16)

    sels = [
        [np.flatnonzero((core_of == c) & (half_of == h)) for h in range(2)]
        for c in range(NCORES)
    ]
    maxb = max(max(len(s) for s in hs) for hs in sels)
    caph = max(((maxb + P - 1) // P) * P, P)

    in_maps = []
    for c in range(NCORES):
        idx_arr = np.stack(
            [_pack_idx(loc[sels[c][h]], caph) for h in range(2)], axis=0
        )
        in_maps.append(
            {
                "comb": comb[c * VS : (c + 1) * VS],
                "attn": attn_r,
                "idx": idx_arr,
            }
        )
    return caph, sels, in_maps, ids_in


def kernel(input_ids, sw, mask, aw, atten, from_kb):
    global LAST_EXEC_TIME_NS, LAST_RESULTS
    from concourse.bass_utils import run_bass_kernel_spmd

    caph, sels, in_maps, ids_in = _prepare(input_ids, sw, mask, aw, atten, from_kb)
    ids = ids_in.reshape(-1)
    n_tok = ids.shape[0]

    if caph not in _PROG_CACHE:
        _PROG_CACHE[caph] = _build_program(caph)
    nc = _PROG_CACHE[caph]

    res = run_bass_kernel_spmd(nc, in_maps, core_ids=list(range(NCORES)))
    LAST_EXEC_TIME_NS = getattr(res, "exec_time_ns", None)
    LAST_RESULTS = res

    full = np.empty((n_tok, D), dtype=np.float32)
    for c in range(NCORES):
        o = res.results[c]["out"]
        for h in range(2):
            sel = sels[c][h]
            if len(sel):
                full[sel] = o[h * caph : h * caph + len(sel)]
    return full.reshape(*ids_in.shape, D)
